# revision 52
# baseline (speedup 1.0000x reference)
"""Block-sparse attention (CABAttention) Trainium2 kernel.

Sharding: 8 cores = 2 batches x 4 head-groups (4 heads each).
Per core: qkv projection (fp16), top-2+diag block-sparse attention
(fp16 values path, fp32 PSUM/softmax-denominator), output projection.
Block selection (top-2 of coarse block-mean scores) is computed on host
in float64 (the PE's fp16 matmul error would flip near-tied blocks)
and passed as index inputs, consumed on device via dynamic
access-pattern offsets on the matmul moving operand.
Unshard: partial output projections (fp16) summed over the 4 cores per
batch (row-parallel tensor split) in fp32 on host, plus bias.

Engine budget (vs the previous fp32r version): input DMAs are token-
sliced and dual-queued so the first qkv matmul group is ready ~4us in;
the v key-major transpose is interleaved into the qkv phase; PSUM->SBUF
copies are split across DVE and Act; the softmax normalize runs on the
(otherwise idle) Pool engine, which has no PSUM port but this multiply
is SBUF-only; the output projection is interleaved into the attention
loop per 128-token tile and written as fp16.
"""
import sys

sys.path.insert(0, "/opt/trn_rl_repo")

import numpy as np

import concourse.bass as bass
import concourse.mybir as mybir
import concourse.tile as tile
from concourse import bacc
from concourse.bass import ds
from concourse.bass_utils import run_bass_kernel_spmd
from concourse.masks import make_identity

F32 = mybir.dt.float32
F16 = mybir.dt.float16
I32 = mybir.dt.int32

DIM = 1024
H = 16
HD = 64
BS = 64
N = 2048
B = 2
M = N // BS            # 32 blocks
SCALE = HD ** -0.5
NCORES = 8
HPC = H // (NCORES // B)   # 4 heads per core

_NC_CACHE = None
LAST_RESULTS = None


def build_kernel():
    nc = bacc.Bacc(None)
    xt_d = nc.dram_tensor("xt", [DIM, N], F16, kind="ExternalInput")
    wq_d = nc.dram_tensor("wq", [DIM, 768], F16, kind="ExternalInput")
    pw_d = nc.dram_tensor("pw", [256, DIM], F16, kind="ExternalInput")
    idx_d = nc.dram_tensor("selidx", [1, 256], I32, kind="ExternalInput")
    wb_d = nc.dram_tensor("wbias", [128, 64], F32, kind="ExternalInput")
    y_d = nc.dram_tensor("y", [N, DIM], F16, kind="ExternalOutput")

    with tile.TileContext(nc) as tc:
        with tc.tile_pool(name="big", bufs=1) as big, \
             tc.tile_pool(name="wrk", bufs=6) as wrk:

            # ---- persistent SBUF tensors ----
            xt = big.tile([128, 8, N], F16)           # x^T, feature-major
            wq = big.tile([128, 8, 768], F16)         # qkv weights^T
            pwt = big.tile([128, 2, DIM], F16)        # proj weights
            idx = big.tile([1, 256], I32)
            wb = big.tile([128, 64], F32)
            qT = [big.tile([128, N], F16, name=f"qT{i}") for i in range(2)]
            kkT = [big.tile([128, N], F16, name=f"kkT{i}") for i in range(2)]
            vvT = [big.tile([128, N], F16, name=f"vvT{i}") for i in range(2)]
            vd = [big.tile([64, 2, N], F16, name=f"vd{i}") for i in range(2)]
            outT = [big.tile([128, N], F16, name=f"outT{i}") for i in range(2)]
            qTB = [big.tile([64, N], F16, name=f"qTB{i}") for i in range(2)]
            kkTB = [big.tile([64, N], F16, name=f"kkTB{i}") for i in range(2)]
            identf = big.tile([128, 128], F32)
            ident = big.tile([128, 128], F16)

            make_identity(nc, identf[:])
            nc.vector.tensor_copy(ident[:], identf[:])

            # ---- input DMAs, token-sliced for early compute start ----
            xt_v = xt_d[:].rearrange("(a p) n -> p a n", p=128)
            wq_v = wq_d[:].rearrange("(a p) n -> p a n", p=128)
            pw_v = pw_d[:].rearrange("(a p) n -> p a n", p=128)

            # qkv weights in three 256-col blocks, v first to match the
            # v -> k -> q matmul order below.
            XSL = [(0, 512), (512, 512), (1024, 512), (1536, 512)]
            nc.sync.dma_start(wq[:, :, 512:768], wq_v[:, :, 512:768])
            for si, (t0, tw) in enumerate(XSL):
                ts_ = slice(t0, t0 + tw)
                for k in range(8):
                    eng = nc.sync if k % 2 == 0 else nc.scalar
                    eng.dma_start(xt[:, k, ts_], xt_v[:, k, ts_])
                if si == 0:
                    nc.sync.dma_start(wq[:, :, 256:512], wq_v[:, :, 256:512])
                elif si == 1:
                    nc.sync.dma_start(wq[:, :, 0:256], wq_v[:, :, 0:256])
                elif si == 2:
                    nc.scalar.dma_start(pwt[:], pw_v[:])
            nc.sync.dma_start(idx[:], idx_d[:])
            nc.sync.dma_start(wb[:], wb_d[:])

            # ---- qkv projection + v key-major transpose, per token slice --
            # v first, then k, then q (matches the weight DMA order above).
            # wq col blocks: q at mt 0,1; k at 2,3; v at 4,5.
            tgt = [(4, vvT[0]), (5, vvT[1]), (2, kkT[0]), (3, kkT[1]),
                   (0, qT[0]), (1, qT[1])]
            cp_state = [0]

            def qkv_group(pool, ts_, mt, dst_t):
                ps = pool.tile([128, 512], F32, name="yp")
                pv = ps[:, 0:ts_.stop - ts_.start]
                for k in range(8):
                    nc.tensor.matmul(
                        pv, lhsT=wq[:, k, mt * 128:(mt + 1) * 128],
                        rhs=xt[:, k, ts_],
                        start=(k == 0), stop=(k == 7))
                if cp_state[0] % 2 == 0:
                    nc.vector.tensor_copy(dst_t[:, ts_], pv)
                else:
                    nc.scalar.copy(dst_t[:, ts_], pv)
                cp_state[0] += 1

            with tc.tile_pool(name="qkps", bufs=4, space="PSUM") as qkps, \
                 tc.tile_pool(name="vtps", bufs=4, space="PSUM") as vtps:
                for t0, tw in XSL:
                    ts_ = slice(t0, t0 + tw)
                    for mt, dst_t in tgt:
                        qkv_group(qkps, ts_, mt, dst_t)
                        if mt == 5:
                            # v done for slice: transpose its blocks
                            for p in range(2):
                                for jb in range(tw // 64):
                                    j = t0 // 64 + jb
                                    tp = vtps.tile([64, 128], F16)
                                    nc.tensor.transpose(
                                        tp[:],
                                        vvT[p][:, j * 64:(j + 1) * 64],
                                        ident[:])
                                    vdst = vd[p][:, :, j * 64:(j + 1) * 64]
                                    src = tp[:].rearrange(
                                        "p (h x) -> p h x", h=2)
                                    if cp_state[0] % 2 == 0:
                                        nc.vector.tensor_copy(vdst, src)
                                    else:
                                        nc.scalar.copy(vdst, src)
                                    cp_state[0] += 1
                    # replicate head-B halves to partition base 0 (dynamic-
                    # offset matmul operands must have partition base 0)
                    for p in range(2):
                        nc.sync.dma_start(kkTB[p][:, ts_],
                                          kkT[p][64:128, ts_])
                        nc.sync.dma_start(qTB[p][:, ts_],
                                          qT[p][64:128, ts_])

            # ---- block-sparse attention + interleaved output projection --
            # Both head-pairs share one softmax pass per query block:
            # sps/pu/pr are [128, 384] with pair p at cols p*192.
            with tc.tile_pool(name="spsp", bufs=2, space="PSUM") as spsp, \
                 tc.tile_pool(name="ptps", bufs=2, space="PSUM") as ptps, \
                 tc.tile_pool(name="avps", bufs=2, space="PSUM") as avps, \
                 tc.tile_pool(name="otps", bufs=1, space="PSUM") as otps, \
                 tc.tile_pool(name="ypsp", bufs=1, space="PSUM") as ypsp:
                anchors = []
                gi_state = [0]

                def emit_jj(jj):
                    gi = gi_state[0]
                    avpP = [avps.tile([128, 128], F32, name=f"avp{p}",
                                      tag=f"av{p}", bufs=1)
                            for p in range(2)]
                    for qb2 in range(2):
                        qb = 2 * jj + qb2
                        qs = slice(qb * 64, (qb + 1) * 64)
                        sps = spsp.tile([128, 384], F32)
                        offs_p = []
                        for p in range(2):
                            base = p * 128 + qb * 4
                            eng = nc.tensor
                            tmps = [eng.alloc_register(f"off_{base + c}")
                                    for c in range(4)]
                            li = eng.reg_load(tmps, idx[0:1, base:base + 4])
                            if gi >= 13:
                                tile.add_dep_helper(
                                    li.ins, anchors[gi - 13].ins,
                                    sync=False,
                                    reason="bound PE register live range")
                            offs = [eng.snap(t_, donate=True, min_val=0,
                                             max_val=N - 64) for t_ in tmps]
                            offs_p.append(offs)
                            gi += 1
                            # scores: slots s0,s1 dynamic; s2 = diag (static)
                            rhs_off = [offs[0], offs[1], qb * 64]
                            rhs_off_b = [offs[2], offs[3], qb * 64]
                            for s in range(3):
                                cs = slice(p * 192 + s * 64,
                                           p * 192 + (s + 1) * 64)
                                nc.tensor.matmul(
                                    sps[0:64, cs], lhsT=qT[p][0:64, qs],
                                    rhs=kkT[p][0:64, ds(rhs_off[s], 64)],
                                    start=True, stop=True,
                                    skip_group_check=True)
                                nc.tensor.matmul(
                                    sps[64:128, cs], lhsT=qTB[p][:, qs],
                                    rhs=kkTB[p][:, ds(rhs_off_b[s], 64)],
                                    start=True, stop=True,
                                    skip_group_check=True,
                                    tile_position=(0, 64))
                        # exp (unnormalized, unmasked)
                        pu = wrk.tile([128, 384], F16, tag="pu")
                        nc.scalar.activation(
                            pu[:], sps[:],
                            mybir.ActivationFunctionType.Exp)
                        # mask duplicated diag slot (x{1,0}, on Pool)
                        for p in range(2):
                            nc.gpsimd.tensor_scalar(
                                pu[:, p * 192 + 128:p * 192 + 192],
                                pu[:, p * 192 + 128:p * 192 + 192],
                                wb[:, p * 32 + qb:p * 32 + qb + 1], None,
                                op0=mybir.AluOpType.mult)
                        # rowsum + reciprocal
                        den = wrk.tile([128, 2], F32, tag="den")
                        for p in range(2):
                            nc.vector.tensor_reduce(
                                den[:, p:p + 1], pu[:, p * 192:(p + 1) * 192],
                                mybir.AxisListType.X, mybir.AluOpType.add)
                        rden = wrk.tile([128, 2], F32, tag="rden")
                        nc.vector.reciprocal(rden[:], den[:])
                        # normalize on Pool (SBUF-only; Pool has no PSUM port)
                        pr = wrk.tile([128, 384], F16, tag="pr")
                        for p in range(2):
                            nc.gpsimd.tensor_scalar(
                                pr[:, p * 192:(p + 1) * 192],
                                pu[:, p * 192:(p + 1) * 192],
                                rden[:, p:p + 1], None,
                                op0=mybir.AluOpType.mult)
                        # transpose probs per slot: [128q,64k]->[64k,128q]
                        pt = ptps.tile([64, 768], F16)
                        for p in range(2):
                            for s in range(3):
                                nc.tensor.transpose(
                                    pt[:, p * 384 + s * 128:
                                       p * 384 + (s + 1) * 128],
                                    pr[:, p * 192 + s * 64:
                                       p * 192 + (s + 1) * 64], ident[:])
                        pts = wrk.tile([64, 768], F16, tag="pts")
                        nc.vector.tensor_copy(pts[:], pt[:])
                        # AV accumulated over slots (dynamic rhs);
                        # qb even -> psum rows 0:64, odd -> 64:128
                        tp_ = None if qb2 == 0 else (0, 64)
                        for p in range(2):
                            avp = avpP[p][qb2 * 64:(qb2 + 1) * 64, :]
                            offs = offs_p[p]
                            ao = [offs[0], offs[1], qb * 64]
                            ao_b = [offs[2], offs[3], qb * 64]
                            for s in range(3):
                                nc.tensor.matmul(
                                    avp[:, 0:64],
                                    lhsT=pts[:, p * 384 + s * 128:
                                             p * 384 + s * 128 + 64],
                                    rhs=vd[p][:, 0, ds(ao[s], 64)],
                                    start=(s == 0), stop=(s == 2),
                                    skip_group_check=True,
                                    tile_position=tp_)
                            for s in range(3):
                                mi = nc.tensor.matmul(
                                    avp[:, 64:128],
                                    lhsT=pts[:, p * 384 + s * 128 + 64:
                                             p * 384 + s * 128 + 128],
                                    rhs=vd[p][:, 1, ds(ao_b[s], 64)],
                                    start=(s == 0), stop=(s == 2),
                                    skip_group_check=True,
                                    tile_position=tp_)
                            anchors.append(mi)
                    # both qbs done: copy AV pairs, transpose
                    # [128tok,128feat] -> [128feat,128tok] per pair
                    otp = otps.tile([128, 256], F16)
                    for p in range(2):
                        av_sb = wrk.tile([128, 128], F16, tag="av_sb")
                        nc.scalar.copy(av_sb[:], avpP[p])
                        nc.tensor.transpose(
                            otp[:, p * 128:(p + 1) * 128], av_sb[:],
                            ident[:])
                    nc.vector.tensor_copy(
                        outT[0][:, jj * 128:(jj + 1) * 128], otp[:, 0:128])
                    nc.vector.tensor_copy(
                        outT[1][:, jj * 128:(jj + 1) * 128], otp[:, 128:256])
                    # output projection for this 128-token tile
                    tks = slice(jj * 128, (jj + 1) * 128)
                    for half in range(2):
                        ns = slice(half * 512, (half + 1) * 512)
                        yp = ypsp.tile([128, 512], F32)
                        nc.tensor.matmul(yp[:], lhsT=outT[0][:, tks],
                                         rhs=pwt[:, 0, ns],
                                         start=True, stop=False)
                        nc.tensor.matmul(yp[:], lhsT=outT[1][:, tks],
                                         rhs=pwt[:, 1, ns],
                                         start=False, stop=True)
                        ys = wrk.tile([128, 512], F16, tag="ys")
                        nc.scalar.copy(ys[:], yp[:])
                        nc.sync.dma_start(y_d[tks, ns], ys[:])
                    gi_state[0] = gi

                for jj in range(16):
                    emit_jj(jj)

    nc.finalize()
    return nc


def _host_prep(x, qkv_w, proj_w):
    """Per-core input maps + block selection (float64, matches fp32 ref)."""
    in_maps = []
    x64 = x.astype(np.float64)
    for core in range(NCORES):
        b = core // (NCORES // B)
        hg = core % (NCORES // B)
        heads = [hg * HPC + i for i in range(HPC)]

        xt = np.ascontiguousarray(x[b].T).astype(np.float16)

        wqkvT = np.empty((DIM, 768), np.float32)
        for p in range(2):
            hA, hB = heads[2 * p], heads[2 * p + 1]
            # q: cols [p*128, p*128+128); k: 256 + p*128; v: 512 + p*128
            wqkvT[:, p*128:p*128+64] = qkv_w[hA*64:(hA+1)*64].T * SCALE
            wqkvT[:, p*128+64:p*128+128] = qkv_w[hB*64:(hB+1)*64].T * SCALE
            kbase = 256 + p * 128
            wqkvT[:, kbase:kbase+64] = qkv_w[DIM+hA*64:DIM+(hA+1)*64].T
            wqkvT[:, kbase+64:kbase+128] = qkv_w[DIM+hB*64:DIM+(hB+1)*64].T
            vbase = 512 + p * 128
            wqkvT[:, vbase:vbase+64] = qkv_w[2*DIM+hA*64:2*DIM+(hA+1)*64].T
            wqkvT[:, vbase+64:vbase+128] = qkv_w[2*DIM+hB*64:2*DIM+(hB+1)*64].T
        wqkvT = wqkvT.astype(np.float16)

        pw = np.ascontiguousarray(
            proj_w[:, heads[0]*64:(heads[-1]+1)*64].T).astype(np.float16)

        # float64 selection (matches fp32 reference ordering w/ margin)
        xb = x64[b].reshape(M, BS, DIM).mean(axis=1)
        selidx = np.zeros((1, 256), np.int32)
        wbias = np.ones((128, 64), np.float32)
        for p in range(2):
            for hip in range(2):
                h = heads[2 * p + hip]
                qb_ = xb @ qkv_w[h*64:(h+1)*64].T.astype(np.float64)
                kb_ = xb @ qkv_w[DIM+h*64:DIM+(h+1)*64].T.astype(np.float64)
                c = qb_ @ kb_.T
                for i in range(M):
                    order = np.argsort(-c[i], kind="stable")
                    i1, i2 = int(order[0]), int(order[1])
                    col = p * 128 + i * 4 + hip * 2
                    selidx[0, col] = i1 * 64
                    selidx[0, col + 1] = i2 * 64
                    if i == i1 or i == i2:
                        wbias[hip*64:(hip+1)*64, p*32+i] = 0.0
        in_maps.append({"xt": xt, "wq": wqkvT, "pw": pw,
                        "selidx": selidx, "wbias": wbias})
    return in_maps


def kernel(x, qkv_w, proj_w, proj_b):
    global _NC_CACHE, LAST_RESULTS
    x = np.asarray(x, np.float32)
    qkv_w = np.asarray(qkv_w, np.float32)
    proj_w = np.asarray(proj_w, np.float32)
    proj_b = np.asarray(proj_b, np.float32)

    if _NC_CACHE is None:
        _NC_CACHE = build_kernel()
    nc = _NC_CACHE

    in_maps = _host_prep(x, qkv_w, proj_w)
    res = run_bass_kernel_spmd(nc, in_maps, list(range(NCORES)))
    LAST_RESULTS = res

    out = np.zeros((B, N, DIM), np.float32)
    for core in range(NCORES):
        out[core // (NCORES // B)] += res.results[core]["y"].astype(np.float32)
    out += proj_b[None, None, :]
    return out


# revision 59
# speedup vs baseline: 1.0185x; 1.0185x over previous
"""Block-sparse attention (CABAttention) Trainium2 kernel.

Sharding: 8 cores = 2 batches x 4 head-groups (4 heads each).
Per core: qkv projection (fp16), top-2+diag block-sparse attention
(fp16 values path, fp32 PSUM/softmax-denominator), output projection.
Block selection (top-2 of coarse block-mean scores) is computed on host
in float64 (the PE's fp16 matmul error would flip near-tied blocks)
and passed as index inputs, consumed on device via dynamic
access-pattern offsets on the matmul moving operand.
Unshard: partial output projections (fp16) summed over the 4 cores per
batch (row-parallel tensor split) in fp32 on host, plus bias.

Engine budget (vs the previous fp32r version): input DMAs are token-
sliced and dual-queued so the first qkv matmul group is ready ~4us in;
the v key-major transpose is interleaved into the qkv phase; PSUM->SBUF
copies are split across DVE and Act; the softmax normalize runs on the
(otherwise idle) Pool engine, which has no PSUM port but this multiply
is SBUF-only; the output projection is interleaved into the attention
loop per 128-token tile and written as fp16.
"""
import sys

sys.path.insert(0, "/opt/trn_rl_repo")

import numpy as np

import concourse.bass as bass
import concourse.mybir as mybir
import concourse.tile as tile
from concourse import bacc
from concourse.bass import ds
from concourse.bass_utils import run_bass_kernel_spmd
from concourse.masks import make_identity

F32 = mybir.dt.float32
F16 = mybir.dt.float16
I32 = mybir.dt.int32

DIM = 1024
H = 16
HD = 64
BS = 64
N = 2048
B = 2
M = N // BS            # 32 blocks
SCALE = HD ** -0.5
NCORES = 8
HPC = H // (NCORES // B)   # 4 heads per core

_NC_CACHE = None
LAST_RESULTS = None


def build_kernel():
    nc = bacc.Bacc(None)
    xt_d = nc.dram_tensor("xt", [DIM, N], F16, kind="ExternalInput")
    wq_d = nc.dram_tensor("wq", [DIM, 768], F16, kind="ExternalInput")
    pw_d = nc.dram_tensor("pw", [256, DIM], F16, kind="ExternalInput")
    idx_d = nc.dram_tensor("selidx", [1, 256], I32, kind="ExternalInput")
    wb_d = nc.dram_tensor("wbias", [128, 64], F32, kind="ExternalInput")
    y_d = nc.dram_tensor("y", [N, DIM], F16, kind="ExternalOutput")

    with tile.TileContext(nc) as tc:
        with tc.tile_pool(name="big", bufs=1) as big, \
             tc.tile_pool(name="wrk", bufs=6) as wrk:

            # ---- persistent SBUF tensors ----
            xt = big.tile([128, 8, N], F16)           # x^T, feature-major
            wq = big.tile([128, 8, 768], F16)         # qkv weights^T
            pwt = big.tile([128, 2, DIM], F16)        # proj weights
            idx = big.tile([1, 256], I32)
            wb = big.tile([128, 64], F32)
            qT = [big.tile([128, N], F16, name=f"qT{i}") for i in range(2)]
            kkT = [big.tile([128, N], F16, name=f"kkT{i}") for i in range(2)]
            vvT = [big.tile([128, N], F16, name=f"vvT{i}") for i in range(2)]
            vd = [big.tile([64, 2, N], F16, name=f"vd{i}") for i in range(2)]
            outT = [big.tile([128, N], F16, name=f"outT{i}") for i in range(2)]
            qTB = [big.tile([64, N], F16, name=f"qTB{i}") for i in range(2)]
            kkTB = [big.tile([64, N], F16, name=f"kkTB{i}") for i in range(2)]
            identf = big.tile([128, 128], F32)
            ident = big.tile([128, 128], F16)

            make_identity(nc, identf[:])
            nc.vector.tensor_copy(ident[:], identf[:])

            # ---- input DMAs, token-sliced for early compute start ----
            xt_v = xt_d[:].rearrange("(a p) n -> p a n", p=128)
            wq_v = wq_d[:].rearrange("(a p) n -> p a n", p=128)
            pw_v = pw_d[:].rearrange("(a p) n -> p a n", p=128)

            # qkv weights in three 256-col blocks, v first to match the
            # v -> k -> q matmul order below.
            XSL = [(0, 512), (512, 512), (1024, 512), (1536, 512)]
            nc.sync.dma_start(wq[:, :, 512:768], wq_v[:, :, 512:768])
            for si, (t0, tw) in enumerate(XSL):
                ts_ = slice(t0, t0 + tw)
                for k in range(8):
                    eng = nc.sync if k % 2 == 0 else nc.scalar
                    eng.dma_start(xt[:, k, ts_], xt_v[:, k, ts_])
                if si == 0:
                    nc.sync.dma_start(wq[:, :, 256:512], wq_v[:, :, 256:512])
                elif si == 1:
                    nc.sync.dma_start(wq[:, :, 0:256], wq_v[:, :, 0:256])
                elif si == 2:
                    nc.scalar.dma_start(pwt[:], pw_v[:])
            nc.sync.dma_start(idx[:], idx_d[:])
            nc.sync.dma_start(wb[:], wb_d[:])

            # ---- qkv projection + v key-major transpose, per token slice --
            # v first, then k, then q (matches the weight DMA order above).
            # wq col blocks: q at mt 0,1; k at 2,3; v at 4,5.
            tgt = [(4, vvT[0]), (5, vvT[1]), (2, kkT[0]), (3, kkT[1]),
                   (0, qT[0]), (1, qT[1])]
            cp_state = [0]

            def qkv_group(pool, ts_, mt, dst_t):
                ps = pool.tile([128, 512], F32, name="yp")
                pv = ps[:, 0:ts_.stop - ts_.start]
                for k in range(8):
                    nc.tensor.matmul(
                        pv, lhsT=wq[:, k, mt * 128:(mt + 1) * 128],
                        rhs=xt[:, k, ts_],
                        start=(k == 0), stop=(k == 7))
                if cp_state[0] % 2 == 0:
                    nc.vector.tensor_copy(dst_t[:, ts_], pv)
                else:
                    nc.scalar.copy(dst_t[:, ts_], pv)
                cp_state[0] += 1

            with tc.tile_pool(name="qkps", bufs=4, space="PSUM") as qkps, \
                 tc.tile_pool(name="vtps", bufs=4, space="PSUM") as vtps:
                # warm the PE p-state during the initial DMA wait: ~2.7us of
                # dummy transposes ramp the clock 0.65->2.4 GHz so the real
                # matmuls start at full speed
                for w in range(20):
                    tp = vtps.tile([64, 128], F16, name="tp")
                    nc.tensor.transpose(tp[:], ident[:, 0:64], ident[:])
                for t0, tw in XSL:
                    ts_ = slice(t0, t0 + tw)
                    for mt, dst_t in tgt:
                        qkv_group(qkps, ts_, mt, dst_t)
                        if mt == 5:
                            # v done for slice: transpose its blocks
                            for p in range(2):
                                for jb in range(tw // 64):
                                    j = t0 // 64 + jb
                                    tp = vtps.tile([64, 128], F16)
                                    nc.tensor.transpose(
                                        tp[:],
                                        vvT[p][:, j * 64:(j + 1) * 64],
                                        ident[:])
                                    vdst = vd[p][:, :, j * 64:(j + 1) * 64]
                                    src = tp[:].rearrange(
                                        "p (h x) -> p h x", h=2)
                                    if cp_state[0] % 2 == 0:
                                        nc.vector.tensor_copy(vdst, src)
                                    else:
                                        nc.scalar.copy(vdst, src)
                                    cp_state[0] += 1
                    # replicate head-B halves to partition base 0 (dynamic-
                    # offset matmul operands must have partition base 0)
                    for p in range(2):
                        nc.sync.dma_start(kkTB[p][:, ts_],
                                          kkT[p][64:128, ts_])
                        nc.sync.dma_start(qTB[p][:, ts_],
                                          qT[p][64:128, ts_])

            # ---- block-sparse attention + interleaved output projection --
            # Both head-pairs share one softmax pass per query block:
            # sps/pu/pr are [128, 384] with pair p at cols p*192.
            with tc.tile_pool(name="spsp", bufs=2, space="PSUM") as spsp, \
                 tc.tile_pool(name="ptps", bufs=2, space="PSUM") as ptps, \
                 tc.tile_pool(name="avps", bufs=2, space="PSUM") as avps, \
                 tc.tile_pool(name="otps", bufs=1, space="PSUM") as otps, \
                 tc.tile_pool(name="ypsp", bufs=1, space="PSUM") as ypsp:
                anchors = []
                gi_state = [0]

                def emit_jj(jj):
                    gi = gi_state[0]
                    avpP = [avps.tile([128, 128], F32, name=f"avp{p}",
                                      tag=f"av{p}", bufs=1)
                            for p in range(2)]
                    for qb2 in range(2):
                        qb = 2 * jj + qb2
                        qs = slice(qb * 64, (qb + 1) * 64)
                        sps = spsp.tile([128, 384], F32)
                        offs_p = []
                        for p in range(2):
                            base = p * 128 + qb * 4
                            eng = nc.tensor
                            tmps = [eng.alloc_register(f"off_{base + c}")
                                    for c in range(4)]
                            li = eng.reg_load(tmps, idx[0:1, base:base + 4])
                            if gi >= 13:
                                tile.add_dep_helper(
                                    li.ins, anchors[gi - 13].ins,
                                    sync=False,
                                    reason="bound PE register live range")
                            offs = [eng.snap(t_, donate=True, min_val=0,
                                             max_val=N - 64) for t_ in tmps]
                            offs_p.append(offs)
                            gi += 1
                            # scores: slots s0,s1 dynamic; s2 = diag (static)
                            rhs_off = [offs[0], offs[1], qb * 64]
                            rhs_off_b = [offs[2], offs[3], qb * 64]
                            for s in range(3):
                                cs = slice(p * 192 + s * 64,
                                           p * 192 + (s + 1) * 64)
                                nc.tensor.matmul(
                                    sps[0:64, cs], lhsT=qT[p][0:64, qs],
                                    rhs=kkT[p][0:64, ds(rhs_off[s], 64)],
                                    start=True, stop=True,
                                    skip_group_check=True)
                                nc.tensor.matmul(
                                    sps[64:128, cs], lhsT=qTB[p][:, qs],
                                    rhs=kkTB[p][:, ds(rhs_off_b[s], 64)],
                                    start=True, stop=True,
                                    skip_group_check=True,
                                    tile_position=(0, 64))
                        # exp (unnormalized, unmasked)
                        pu = wrk.tile([128, 384], F16, tag="pu")
                        nc.scalar.activation(
                            pu[:], sps[:],
                            mybir.ActivationFunctionType.Exp)
                        # mask duplicated diag slot (x{1,0}, on Pool)
                        for p in range(2):
                            nc.gpsimd.tensor_scalar(
                                pu[:, p * 192 + 128:p * 192 + 192],
                                pu[:, p * 192 + 128:p * 192 + 192],
                                wb[:, p * 32 + qb:p * 32 + qb + 1], None,
                                op0=mybir.AluOpType.mult)
                        # rowsum + reciprocal
                        den = wrk.tile([128, 2], F32, tag="den")
                        for p in range(2):
                            nc.vector.tensor_reduce(
                                den[:, p:p + 1], pu[:, p * 192:(p + 1) * 192],
                                mybir.AxisListType.X, mybir.AluOpType.add)
                        rden = wrk.tile([128, 2], F32, tag="rden")
                        nc.vector.reciprocal(rden[:], den[:])
                        # normalize on Pool (SBUF-only; Pool has no PSUM port)
                        pr = wrk.tile([128, 384], F16, tag="pr")
                        for p in range(2):
                            nc.gpsimd.tensor_scalar(
                                pr[:, p * 192:(p + 1) * 192],
                                pu[:, p * 192:(p + 1) * 192],
                                rden[:, p:p + 1], None,
                                op0=mybir.AluOpType.mult)
                        # transpose probs per slot: [128q,64k]->[64k,128q]
                        pt = ptps.tile([64, 768], F16)
                        for p in range(2):
                            for s in range(3):
                                nc.tensor.transpose(
                                    pt[:, p * 384 + s * 128:
                                       p * 384 + (s + 1) * 128],
                                    pr[:, p * 192 + s * 64:
                                       p * 192 + (s + 1) * 64], ident[:])
                        pts = wrk.tile([64, 768], F16, tag="pts")
                        nc.vector.tensor_copy(pts[:], pt[:])
                        # AV accumulated over slots (dynamic rhs);
                        # qb even -> psum rows 0:64, odd -> 64:128
                        tp_ = None if qb2 == 0 else (0, 64)
                        for p in range(2):
                            avp = avpP[p][qb2 * 64:(qb2 + 1) * 64, :]
                            offs = offs_p[p]
                            ao = [offs[0], offs[1], qb * 64]
                            ao_b = [offs[2], offs[3], qb * 64]
                            for s in range(3):
                                nc.tensor.matmul(
                                    avp[:, 0:64],
                                    lhsT=pts[:, p * 384 + s * 128:
                                             p * 384 + s * 128 + 64],
                                    rhs=vd[p][:, 0, ds(ao[s], 64)],
                                    start=(s == 0), stop=(s == 2),
                                    skip_group_check=True,
                                    tile_position=tp_)
                            for s in range(3):
                                mi = nc.tensor.matmul(
                                    avp[:, 64:128],
                                    lhsT=pts[:, p * 384 + s * 128 + 64:
                                             p * 384 + s * 128 + 128],
                                    rhs=vd[p][:, 1, ds(ao_b[s], 64)],
                                    start=(s == 0), stop=(s == 2),
                                    skip_group_check=True,
                                    tile_position=tp_)
                            anchors.append(mi)
                    # both qbs done: copy AV pairs, transpose
                    # [128tok,128feat] -> [128feat,128tok] per pair
                    otp = otps.tile([128, 256], F16)
                    for p in range(2):
                        av_sb = wrk.tile([128, 128], F16, tag="av_sb")
                        nc.scalar.copy(av_sb[:], avpP[p])
                        nc.tensor.transpose(
                            otp[:, p * 128:(p + 1) * 128], av_sb[:],
                            ident[:])
                    nc.vector.tensor_copy(
                        outT[0][:, jj * 128:(jj + 1) * 128], otp[:, 0:128])
                    nc.vector.tensor_copy(
                        outT[1][:, jj * 128:(jj + 1) * 128], otp[:, 128:256])
                    # output projection for this 128-token tile
                    tks = slice(jj * 128, (jj + 1) * 128)
                    for half in range(2):
                        ns = slice(half * 512, (half + 1) * 512)
                        yp = ypsp.tile([128, 512], F32)
                        nc.tensor.matmul(yp[:], lhsT=outT[0][:, tks],
                                         rhs=pwt[:, 0, ns],
                                         start=True, stop=False)
                        nc.tensor.matmul(yp[:], lhsT=outT[1][:, tks],
                                         rhs=pwt[:, 1, ns],
                                         start=False, stop=True)
                        ys = wrk.tile([128, 512], F16, tag="ys")
                        nc.scalar.copy(ys[:], yp[:])
                        nc.sync.dma_start(y_d[tks, ns], ys[:])
                    gi_state[0] = gi

                for jj in range(16):
                    emit_jj(jj)

    nc.finalize()
    return nc


def _host_prep(x, qkv_w, proj_w):
    """Per-core input maps + block selection (float64, matches fp32 ref)."""
    in_maps = []
    x64 = x.astype(np.float64)
    for core in range(NCORES):
        b = core // (NCORES // B)
        hg = core % (NCORES // B)
        heads = [hg * HPC + i for i in range(HPC)]

        xt = np.ascontiguousarray(x[b].T).astype(np.float16)

        wqkvT = np.empty((DIM, 768), np.float32)
        for p in range(2):
            hA, hB = heads[2 * p], heads[2 * p + 1]
            # q: cols [p*128, p*128+128); k: 256 + p*128; v: 512 + p*128
            wqkvT[:, p*128:p*128+64] = qkv_w[hA*64:(hA+1)*64].T * SCALE
            wqkvT[:, p*128+64:p*128+128] = qkv_w[hB*64:(hB+1)*64].T * SCALE
            kbase = 256 + p * 128
            wqkvT[:, kbase:kbase+64] = qkv_w[DIM+hA*64:DIM+(hA+1)*64].T
            wqkvT[:, kbase+64:kbase+128] = qkv_w[DIM+hB*64:DIM+(hB+1)*64].T
            vbase = 512 + p * 128
            wqkvT[:, vbase:vbase+64] = qkv_w[2*DIM+hA*64:2*DIM+(hA+1)*64].T
            wqkvT[:, vbase+64:vbase+128] = qkv_w[2*DIM+hB*64:2*DIM+(hB+1)*64].T
        wqkvT = wqkvT.astype(np.float16)

        pw = np.ascontiguousarray(
            proj_w[:, heads[0]*64:(heads[-1]+1)*64].T).astype(np.float16)

        # float64 selection (matches fp32 reference ordering w/ margin)
        xb = x64[b].reshape(M, BS, DIM).mean(axis=1)
        selidx = np.zeros((1, 256), np.int32)
        wbias = np.ones((128, 64), np.float32)
        for p in range(2):
            for hip in range(2):
                h = heads[2 * p + hip]
                qb_ = xb @ qkv_w[h*64:(h+1)*64].T.astype(np.float64)
                kb_ = xb @ qkv_w[DIM+h*64:DIM+(h+1)*64].T.astype(np.float64)
                c = qb_ @ kb_.T
                for i in range(M):
                    order = np.argsort(-c[i], kind="stable")
                    i1, i2 = int(order[0]), int(order[1])
                    col = p * 128 + i * 4 + hip * 2
                    selidx[0, col] = i1 * 64
                    selidx[0, col + 1] = i2 * 64
                    if i == i1 or i == i2:
                        wbias[hip*64:(hip+1)*64, p*32+i] = 0.0
        in_maps.append({"xt": xt, "wq": wqkvT, "pw": pw,
                        "selidx": selidx, "wbias": wbias})
    return in_maps


def kernel(x, qkv_w, proj_w, proj_b):
    global _NC_CACHE, LAST_RESULTS
    x = np.asarray(x, np.float32)
    qkv_w = np.asarray(qkv_w, np.float32)
    proj_w = np.asarray(proj_w, np.float32)
    proj_b = np.asarray(proj_b, np.float32)

    if _NC_CACHE is None:
        _NC_CACHE = build_kernel()
    nc = _NC_CACHE

    in_maps = _host_prep(x, qkv_w, proj_w)
    res = run_bass_kernel_spmd(nc, in_maps, list(range(NCORES)))
    LAST_RESULTS = res

    out = np.zeros((B, N, DIM), np.float32)
    for core in range(NCORES):
        out[core // (NCORES // B)] += res.results[core]["y"].astype(np.float32)
    out += proj_b[None, None, :]
    return out


# revision 63
# speedup vs baseline: 1.0193x; 1.0008x over previous
"""Block-sparse attention (CABAttention) Trainium2 kernel.

Sharding: 8 cores = 2 batches x 4 head-groups (4 heads each).
Per core: qkv projection (fp16), top-2+diag block-sparse attention
(fp16 values path, fp32 PSUM/softmax-denominator), output projection.
Block selection (top-2 of coarse block-mean scores) is computed on host
in float64 (the PE's fp16 matmul error would flip near-tied blocks)
and passed as index inputs, consumed on device via dynamic
access-pattern offsets on the matmul moving operand.
Unshard: partial output projections (fp16) summed over the 4 cores per
batch (row-parallel tensor split) in fp32 on host, plus bias.

Engine budget (vs the previous fp32r version): input DMAs are token-
sliced and dual-queued so the first qkv matmul group is ready ~4us in;
the v key-major transpose is interleaved into the qkv phase; PSUM->SBUF
copies are split across DVE and Act; the softmax normalize runs on the
(otherwise idle) Pool engine, which has no PSUM port but this multiply
is SBUF-only; the output projection is interleaved into the attention
loop per 128-token tile and written as fp16.
"""
import sys

sys.path.insert(0, "/opt/trn_rl_repo")

import numpy as np

import concourse.bass as bass
import concourse.mybir as mybir
import concourse.tile as tile
from concourse import bacc
from concourse.bass import ds
from concourse.bass_utils import run_bass_kernel_spmd
from concourse.masks import make_identity

F32 = mybir.dt.float32
F16 = mybir.dt.float16
I32 = mybir.dt.int32

DIM = 1024
H = 16
HD = 64
BS = 64
N = 2048
B = 2
M = N // BS            # 32 blocks
SCALE = HD ** -0.5
NCORES = 8
HPC = H // (NCORES // B)   # 4 heads per core

_NC_CACHE = None
LAST_RESULTS = None


def build_kernel():
    nc = bacc.Bacc(None)
    xt_d = nc.dram_tensor("xt", [DIM, N], F16, kind="ExternalInput")
    wq_d = nc.dram_tensor("wq", [DIM, 768], F16, kind="ExternalInput")
    pw_d = nc.dram_tensor("pw", [256, DIM], F16, kind="ExternalInput")
    idx_d = nc.dram_tensor("selidx", [1, 256], I32, kind="ExternalInput")
    wb_d = nc.dram_tensor("wbias", [128, 64], F32, kind="ExternalInput")
    y_d = nc.dram_tensor("y", [N, DIM], F16, kind="ExternalOutput")

    with tile.TileContext(nc) as tc:
        with tc.tile_pool(name="big", bufs=1) as big, \
             tc.tile_pool(name="wrk", bufs=6) as wrk:

            # ---- persistent SBUF tensors ----
            xt = big.tile([128, 8, N], F16)           # x^T, feature-major
            wq = big.tile([128, 8, 768], F16)         # qkv weights^T
            pwt = big.tile([128, 2, DIM], F16)        # proj weights
            idx = big.tile([1, 256], I32)
            wb = big.tile([128, 64], F32)
            qT = [big.tile([128, N], F16, name=f"qT{i}") for i in range(2)]
            kkT = [big.tile([128, N], F16, name=f"kkT{i}") for i in range(2)]
            vvT = [big.tile([128, N], F16, name=f"vvT{i}") for i in range(2)]
            vd = [big.tile([64, 2, N], F16, name=f"vd{i}") for i in range(2)]
            outT = [big.tile([128, N], F16, name=f"outT{i}") for i in range(2)]
            qTB = [big.tile([64, N], F16, name=f"qTB{i}") for i in range(2)]
            kkTB = [big.tile([64, N], F16, name=f"kkTB{i}") for i in range(2)]
            identf = big.tile([128, 128], F32)
            ident = big.tile([128, 128], F16)

            make_identity(nc, identf[:])
            nc.vector.tensor_copy(ident[:], identf[:])

            # ---- input DMAs, token-sliced for early compute start ----
            xt_v = xt_d[:].rearrange("(a p) n -> p a n", p=128)
            wq_v = wq_d[:].rearrange("(a p) n -> p a n", p=128)
            pw_v = pw_d[:].rearrange("(a p) n -> p a n", p=128)

            # qkv weights in three 256-col blocks, v first to match the
            # v -> k -> q matmul order below.
            XSL = [(0, 512), (512, 512), (1024, 512), (1536, 512)]
            nc.sync.dma_start(wq[:, :, 512:768], wq_v[:, :, 512:768])
            for si, (t0, tw) in enumerate(XSL):
                ts_ = slice(t0, t0 + tw)
                for k in range(8):
                    eng = nc.sync if k % 2 == 0 else nc.scalar
                    eng.dma_start(xt[:, k, ts_], xt_v[:, k, ts_])
                if si == 0:
                    nc.sync.dma_start(wq[:, :, 256:512], wq_v[:, :, 256:512])
                elif si == 1:
                    nc.sync.dma_start(wq[:, :, 0:256], wq_v[:, :, 0:256])
                elif si == 2:
                    nc.scalar.dma_start(pwt[:], pw_v[:])
            nc.sync.dma_start(idx[:], idx_d[:])
            nc.sync.dma_start(wb[:], wb_d[:])

            # ---- qkv projection + v key-major transpose, per token slice --
            # v first, then k, then q (matches the weight DMA order above).
            # wq col blocks: q at mt 0,1; k at 2,3; v at 4,5.
            tgt = [(4, vvT[0]), (5, vvT[1]), (2, kkT[0]), (3, kkT[1]),
                   (0, qT[0]), (1, qT[1])]
            cp_state = [0]

            def qkv_group(pool, ts_, mt, dst_t):
                ps = pool.tile([128, 512], F32, name="yp")
                pv = ps[:, 0:ts_.stop - ts_.start]
                for k in range(8):
                    nc.tensor.matmul(
                        pv, lhsT=wq[:, k, mt * 128:(mt + 1) * 128],
                        rhs=xt[:, k, ts_],
                        start=(k == 0), stop=(k == 7))
                if cp_state[0] % 2 == 0:
                    nc.vector.tensor_copy(dst_t[:, ts_], pv)
                else:
                    nc.scalar.copy(dst_t[:, ts_], pv)
                cp_state[0] += 1

            with tc.tile_pool(name="qkps", bufs=4, space="PSUM") as qkps, \
                 tc.tile_pool(name="vtps", bufs=4, space="PSUM") as vtps:
                # warm the PE p-state during the initial DMA wait: ~2.7us of
                # dummy transposes ramp the clock 0.65->2.4 GHz so the real
                # matmuls start at full speed
                for w in range(20):
                    tp = vtps.tile([64, 128], F16, name="tp")
                    nc.tensor.transpose(tp[:], ident[:, 0:64], ident[:])
                for t0, tw in XSL:
                    ts_ = slice(t0, t0 + tw)
                    for mt, dst_t in tgt:
                        qkv_group(qkps, ts_, mt, dst_t)
                        if mt == 5:
                            # v done for slice: transpose its blocks
                            for p in range(2):
                                for jb in range(tw // 64):
                                    j = t0 // 64 + jb
                                    tp = vtps.tile([64, 128], F16)
                                    nc.tensor.transpose(
                                        tp[:],
                                        vvT[p][:, j * 64:(j + 1) * 64],
                                        ident[:])
                                    vdst = vd[p][:, :, j * 64:(j + 1) * 64]
                                    src = tp[:].rearrange(
                                        "p (h x) -> p h x", h=2)
                                    if cp_state[0] % 2 == 0:
                                        nc.vector.tensor_copy(vdst, src)
                                    else:
                                        nc.scalar.copy(vdst, src)
                                    cp_state[0] += 1
                    # replicate head-B halves to partition base 0 (dynamic-
                    # offset matmul operands must have partition base 0)
                    for p in range(2):
                        nc.sync.dma_start(kkTB[p][:, ts_],
                                          kkT[p][64:128, ts_])
                        nc.sync.dma_start(qTB[p][:, ts_],
                                          qT[p][64:128, ts_])

            # ---- block-sparse attention + interleaved output projection --
            # Both head-pairs share one softmax pass per query block:
            # sps/pu/pr are [128, 384] with pair p at cols p*192.
            with tc.tile_pool(name="spsp", bufs=3, space="PSUM") as spsp, \
                 tc.tile_pool(name="ptps", bufs=2, space="PSUM") as ptps, \
                 tc.tile_pool(name="avps", bufs=1, space="PSUM") as avps, \
                 tc.tile_pool(name="otps", bufs=1, space="PSUM") as otps, \
                 tc.tile_pool(name="ypsp", bufs=1, space="PSUM") as ypsp:
                anchors = []
                gi_state = [0]

                def emit_jj(jj):
                    gi = gi_state[0]
                    avp2 = avps.tile([128, 256], F32, name="avp2")
                    avpP = [avp2[:, p * 128:(p + 1) * 128] for p in range(2)]
                    for qb2 in range(2):
                        qb = 2 * jj + qb2
                        qs = slice(qb * 64, (qb + 1) * 64)
                        sps = spsp.tile([128, 384], F32)
                        offs_p = []
                        for p in range(2):
                            base = p * 128 + qb * 4
                            eng = nc.tensor
                            tmps = [eng.alloc_register(f"off_{base + c}")
                                    for c in range(4)]
                            li = eng.reg_load(tmps, idx[0:1, base:base + 4])
                            if gi >= 13:
                                tile.add_dep_helper(
                                    li.ins, anchors[gi - 13].ins,
                                    sync=False,
                                    reason="bound PE register live range")
                            offs = [eng.snap(t_, donate=True, min_val=0,
                                             max_val=N - 64) for t_ in tmps]
                            offs_p.append(offs)
                            gi += 1
                            # scores: slots s0,s1 dynamic; s2 = diag (static)
                            rhs_off = [offs[0], offs[1], qb * 64]
                            rhs_off_b = [offs[2], offs[3], qb * 64]
                            for s in range(3):
                                cs = slice(p * 192 + s * 64,
                                           p * 192 + (s + 1) * 64)
                                nc.tensor.matmul(
                                    sps[0:64, cs], lhsT=qT[p][0:64, qs],
                                    rhs=kkT[p][0:64, ds(rhs_off[s], 64)],
                                    start=True, stop=True,
                                    skip_group_check=True)
                                nc.tensor.matmul(
                                    sps[64:128, cs], lhsT=qTB[p][:, qs],
                                    rhs=kkTB[p][:, ds(rhs_off_b[s], 64)],
                                    start=True, stop=True,
                                    skip_group_check=True,
                                    tile_position=(0, 64))
                        # exp (unnormalized, unmasked)
                        pu = wrk.tile([128, 384], F16, tag="pu")
                        nc.scalar.activation(
                            pu[:], sps[:],
                            mybir.ActivationFunctionType.Exp)
                        # mask duplicated diag slot (x{1,0}, on Pool)
                        for p in range(2):
                            nc.gpsimd.tensor_scalar(
                                pu[:, p * 192 + 128:p * 192 + 192],
                                pu[:, p * 192 + 128:p * 192 + 192],
                                wb[:, p * 32 + qb:p * 32 + qb + 1], None,
                                op0=mybir.AluOpType.mult)
                        # rowsum + reciprocal
                        den = wrk.tile([128, 2], F32, tag="den")
                        for p in range(2):
                            nc.vector.tensor_reduce(
                                den[:, p:p + 1], pu[:, p * 192:(p + 1) * 192],
                                mybir.AxisListType.X, mybir.AluOpType.add)
                        rden = wrk.tile([128, 2], F32, tag="rden")
                        nc.vector.reciprocal(rden[:], den[:])
                        # normalize on Pool (SBUF-only; Pool has no PSUM port)
                        pr = wrk.tile([128, 384], F16, tag="pr")
                        for p in range(2):
                            nc.gpsimd.tensor_scalar(
                                pr[:, p * 192:(p + 1) * 192],
                                pu[:, p * 192:(p + 1) * 192],
                                rden[:, p:p + 1], None,
                                op0=mybir.AluOpType.mult)
                        # transpose probs per slot: [128q,64k]->[64k,128q]
                        pt = ptps.tile([64, 768], F16)
                        for p in range(2):
                            for s in range(3):
                                nc.tensor.transpose(
                                    pt[:, p * 384 + s * 128:
                                       p * 384 + (s + 1) * 128],
                                    pr[:, p * 192 + s * 64:
                                       p * 192 + (s + 1) * 64], ident[:])
                        pts = wrk.tile([64, 768], F16, tag="pts")
                        nc.vector.tensor_copy(pts[:], pt[:])
                        # AV accumulated over slots (dynamic rhs);
                        # qb even -> psum rows 0:64, odd -> 64:128
                        tp_ = None if qb2 == 0 else (0, 64)
                        for p in range(2):
                            avp = avpP[p][qb2 * 64:(qb2 + 1) * 64, :]
                            offs = offs_p[p]
                            ao = [offs[0], offs[1], qb * 64]
                            ao_b = [offs[2], offs[3], qb * 64]
                            for s in range(3):
                                nc.tensor.matmul(
                                    avp[:, 0:64],
                                    lhsT=pts[:, p * 384 + s * 128:
                                             p * 384 + s * 128 + 64],
                                    rhs=vd[p][:, 0, ds(ao[s], 64)],
                                    start=(s == 0), stop=(s == 2),
                                    skip_group_check=True,
                                    tile_position=tp_)
                            for s in range(3):
                                mi = nc.tensor.matmul(
                                    avp[:, 64:128],
                                    lhsT=pts[:, p * 384 + s * 128 + 64:
                                             p * 384 + s * 128 + 128],
                                    rhs=vd[p][:, 1, ds(ao_b[s], 64)],
                                    start=(s == 0), stop=(s == 2),
                                    skip_group_check=True,
                                    tile_position=tp_)
                            anchors.append(mi)
                    # both qbs done: copy AV pairs, transpose
                    # [128tok,128feat] -> [128feat,128tok] per pair
                    otp = otps.tile([128, 256], F16)
                    for p in range(2):
                        av_sb = wrk.tile([128, 128], F16, tag="av_sb")
                        nc.scalar.copy(av_sb[:], avpP[p])
                        nc.tensor.transpose(
                            otp[:, p * 128:(p + 1) * 128], av_sb[:],
                            ident[:])
                    nc.vector.tensor_copy(
                        outT[0][:, jj * 128:(jj + 1) * 128], otp[:, 0:128])
                    nc.vector.tensor_copy(
                        outT[1][:, jj * 128:(jj + 1) * 128], otp[:, 128:256])
                    # output projection for this 128-token tile
                    tks = slice(jj * 128, (jj + 1) * 128)
                    for half in range(2):
                        ns = slice(half * 512, (half + 1) * 512)
                        yp = ypsp.tile([128, 512], F32)
                        nc.tensor.matmul(yp[:], lhsT=outT[0][:, tks],
                                         rhs=pwt[:, 0, ns],
                                         start=True, stop=False)
                        nc.tensor.matmul(yp[:], lhsT=outT[1][:, tks],
                                         rhs=pwt[:, 1, ns],
                                         start=False, stop=True)
                        ys = wrk.tile([128, 512], F16, tag="ys")
                        nc.scalar.copy(ys[:], yp[:])
                        nc.sync.dma_start(y_d[tks, ns], ys[:])
                    gi_state[0] = gi

                for jj in range(16):
                    emit_jj(jj)

    nc.finalize()
    return nc


def _host_prep(x, qkv_w, proj_w):
    """Per-core input maps + block selection (float64, matches fp32 ref)."""
    in_maps = []
    x64 = x.astype(np.float64)
    for core in range(NCORES):
        b = core // (NCORES // B)
        hg = core % (NCORES // B)
        heads = [hg * HPC + i for i in range(HPC)]

        xt = np.ascontiguousarray(x[b].T).astype(np.float16)

        wqkvT = np.empty((DIM, 768), np.float32)
        for p in range(2):
            hA, hB = heads[2 * p], heads[2 * p + 1]
            # q: cols [p*128, p*128+128); k: 256 + p*128; v: 512 + p*128
            wqkvT[:, p*128:p*128+64] = qkv_w[hA*64:(hA+1)*64].T * SCALE
            wqkvT[:, p*128+64:p*128+128] = qkv_w[hB*64:(hB+1)*64].T * SCALE
            kbase = 256 + p * 128
            wqkvT[:, kbase:kbase+64] = qkv_w[DIM+hA*64:DIM+(hA+1)*64].T
            wqkvT[:, kbase+64:kbase+128] = qkv_w[DIM+hB*64:DIM+(hB+1)*64].T
            vbase = 512 + p * 128
            wqkvT[:, vbase:vbase+64] = qkv_w[2*DIM+hA*64:2*DIM+(hA+1)*64].T
            wqkvT[:, vbase+64:vbase+128] = qkv_w[2*DIM+hB*64:2*DIM+(hB+1)*64].T
        wqkvT = wqkvT.astype(np.float16)

        pw = np.ascontiguousarray(
            proj_w[:, heads[0]*64:(heads[-1]+1)*64].T).astype(np.float16)

        # float64 selection (matches fp32 reference ordering w/ margin)
        xb = x64[b].reshape(M, BS, DIM).mean(axis=1)
        selidx = np.zeros((1, 256), np.int32)
        wbias = np.ones((128, 64), np.float32)
        for p in range(2):
            for hip in range(2):
                h = heads[2 * p + hip]
                qb_ = xb @ qkv_w[h*64:(h+1)*64].T.astype(np.float64)
                kb_ = xb @ qkv_w[DIM+h*64:DIM+(h+1)*64].T.astype(np.float64)
                c = qb_ @ kb_.T
                for i in range(M):
                    order = np.argsort(-c[i], kind="stable")
                    i1, i2 = int(order[0]), int(order[1])
                    col = p * 128 + i * 4 + hip * 2
                    selidx[0, col] = i1 * 64
                    selidx[0, col + 1] = i2 * 64
                    if i == i1 or i == i2:
                        wbias[hip*64:(hip+1)*64, p*32+i] = 0.0
        in_maps.append({"xt": xt, "wq": wqkvT, "pw": pw,
                        "selidx": selidx, "wbias": wbias})
    return in_maps


def kernel(x, qkv_w, proj_w, proj_b):
    global _NC_CACHE, LAST_RESULTS
    x = np.asarray(x, np.float32)
    qkv_w = np.asarray(qkv_w, np.float32)
    proj_w = np.asarray(proj_w, np.float32)
    proj_b = np.asarray(proj_b, np.float32)

    if _NC_CACHE is None:
        _NC_CACHE = build_kernel()
    nc = _NC_CACHE

    in_maps = _host_prep(x, qkv_w, proj_w)
    res = run_bass_kernel_spmd(nc, in_maps, list(range(NCORES)))
    LAST_RESULTS = res

    out = np.zeros((B, N, DIM), np.float32)
    for core in range(NCORES):
        out[core // (NCORES // B)] += res.results[core]["y"].astype(np.float32)
    out += proj_b[None, None, :]
    return out


# revision 78
# speedup vs baseline: 1.0327x; 1.0131x over previous
"""Block-sparse attention (CABAttention) Trainium2 kernel.

Sharding: 8 cores = 2 batches x 4 head-groups (4 heads each).
Per core: qkv projection (fp16), top-2+diag block-sparse attention
(fp16 values path, fp32 PSUM/softmax-denominator), output projection.
Block selection (top-2 of coarse block-mean scores) is computed on host
in float64 (the PE's fp16 matmul error would flip near-tied blocks)
and passed as index inputs, consumed on device via dynamic
access-pattern offsets on the matmul moving operand.
Unshard: partial output projections (fp16) summed over the 4 cores per
batch (row-parallel tensor split) in fp32 on host, plus bias.

Engine budget (vs the previous fp32r version): input DMAs are token-
sliced and dual-queued so the first qkv matmul group is ready ~4us in;
the v key-major transpose is interleaved into the qkv phase; PSUM->SBUF
copies are split across DVE and Act; the softmax normalize runs on the
(otherwise idle) Pool engine, which has no PSUM port but this multiply
is SBUF-only; the output projection is interleaved into the attention
loop per 128-token tile and written as fp16.
"""
import sys

sys.path.insert(0, "/opt/trn_rl_repo")

import numpy as np

import concourse.bass as bass
import concourse.mybir as mybir
import concourse.tile as tile
from concourse import bacc
from concourse.bass import ds
from concourse.bass_utils import run_bass_kernel_spmd
from concourse.masks import make_identity

F32 = mybir.dt.float32
F16 = mybir.dt.float16
I32 = mybir.dt.int32

DIM = 1024
H = 16
HD = 64
BS = 64
N = 2048
B = 2
M = N // BS            # 32 blocks
SCALE = HD ** -0.5
NCORES = 8
HPC = H // (NCORES // B)   # 4 heads per core

_NC_CACHE = None
LAST_RESULTS = None


def build_kernel():
    nc = bacc.Bacc(None)
    xt_d = nc.dram_tensor("xt", [DIM, N], F16, kind="ExternalInput")
    wq_d = nc.dram_tensor("wq", [DIM, 768], F16, kind="ExternalInput")
    pw_d = nc.dram_tensor("pw", [256, DIM], F16, kind="ExternalInput")
    idx_d = nc.dram_tensor("selidx", [1, 256], I32, kind="ExternalInput")
    wb_d = nc.dram_tensor("wbias", [128, 64], F32, kind="ExternalInput")
    y_d = nc.dram_tensor("y", [N, DIM], F16, kind="ExternalOutput")

    with tile.TileContext(nc) as tc:
        with tc.tile_pool(name="big", bufs=1) as big, \
             tc.tile_pool(name="wrk", bufs=6) as wrk:

            # ---- persistent SBUF tensors ----
            xt = big.tile([128, 8, N], F16)           # x^T, feature-major
            wq = big.tile([128, 8, 768], F16)         # qkv weights^T
            pwt = big.tile([128, 2, DIM], F16)        # proj weights
            idx = big.tile([1, 256], I32)
            wb = big.tile([128, 64], F32)
            qT = [big.tile([128, N], F16, name=f"qT{i}") for i in range(2)]
            kkT = [big.tile([128, N], F16, name=f"kkT{i}") for i in range(2)]
            vvT = [big.tile([128, N], F16, name=f"vvT{i}") for i in range(2)]
            vd = [big.tile([64, 2, N], F16, name=f"vd{i}") for i in range(2)]
            outT = [big.tile([128, N], F16, name=f"outT{i}") for i in range(2)]
            qTB = [big.tile([64, N], F16, name=f"qTB{i}") for i in range(2)]
            kkTB = [big.tile([64, N], F16, name=f"kkTB{i}") for i in range(2)]
            identf = big.tile([128, 128], F32)
            ident = big.tile([128, 128], F16)

            make_identity(nc, identf[:])
            nc.vector.tensor_copy(ident[:], identf[:])

            # ---- input DMAs, token-sliced for early compute start ----
            xt_v = xt_d[:].rearrange("(a p) n -> p a n", p=128)
            wq_v = wq_d[:].rearrange("(a p) n -> p a n", p=128)
            pw_v = pw_d[:].rearrange("(a p) n -> p a n", p=128)

            # qkv weights in three 256-col blocks, v first to match the
            # v -> k -> q matmul order below.
            XSL = [(0, 512), (512, 512), (1024, 512), (1536, 512)]
            nc.sync.dma_start(wq[:, :, 512:768], wq_v[:, :, 512:768])
            for si, (t0, tw) in enumerate(XSL):
                ts_ = slice(t0, t0 + tw)
                for k in range(8):
                    eng = nc.sync if k % 2 == 0 else nc.scalar
                    eng.dma_start(xt[:, k, ts_], xt_v[:, k, ts_])
                if si == 0:
                    nc.sync.dma_start(wq[:, :, 256:512], wq_v[:, :, 256:512])
                elif si == 1:
                    nc.sync.dma_start(wq[:, :, 0:256], wq_v[:, :, 0:256])
            nc.sync.dma_start(idx[:], idx_d[:])
            nc.sync.dma_start(wb[:], wb_d[:])
            nc.scalar.dma_start(pwt[:], pw_v[:])

            # ---- qkv projection + v key-major transpose, per token slice --
            # v first, then k, then q (matches the weight DMA order above).
            # wq col blocks: q at mt 0,1; k at 2,3; v at 4,5.
            tgt = [(4, vvT[0]), (5, vvT[1]), (2, kkT[0]), (3, kkT[1]),
                   (0, qT[0]), (1, qT[1])]
            cp_state = [0]

            def qkv_group(pool, ts_, mt, dst_t):
                ps = pool.tile([128, 512], F32, name="yp")
                pv = ps[:, 0:ts_.stop - ts_.start]
                for k in range(8):
                    nc.tensor.matmul(
                        pv, lhsT=wq[:, k, mt * 128:(mt + 1) * 128],
                        rhs=xt[:, k, ts_],
                        start=(k == 0), stop=(k == 7))
                if cp_state[0] % 2 == 0:
                    nc.vector.tensor_copy(dst_t[:, ts_], pv)
                else:
                    nc.scalar.copy(dst_t[:, ts_], pv)
                cp_state[0] += 1

            anchors = []
            gi_state = [0]
            head_cache = {}
            spsp_ctx = tc.tile_pool(name="spsp", bufs=3, space="PSUM")
            spsp = spsp_ctx.__enter__()

            def emit_head(qb):
                # scores -> exp -> mask -> rowsum -> recip -> normalize;
                # touches only spsp + SBUF, so jj0's heads can be emitted
                # mid qkv-phase to pre-fill the softmax pipeline.
                gi = gi_state[0]
                qs = slice(qb * 64, (qb + 1) * 64)
                sps = spsp.tile([128, 384], F32, name="sps")
                offs_p = []
                for p in range(2):
                    base = p * 128 + qb * 4
                    eng = nc.tensor
                    tmps = [eng.alloc_register(f"off_{base + c}")
                            for c in range(4)]
                    li = eng.reg_load(tmps, idx[0:1, base:base + 4])
                    if gi >= 13:
                        tile.add_dep_helper(
                            li.ins, anchors[gi - 13].ins, sync=False,
                            reason="bound PE register live range")
                    offs = [eng.snap(t_, donate=True, min_val=0,
                                     max_val=N - 64) for t_ in tmps]
                    offs_p.append(offs)
                    gi += 1
                    # scores: slots s0,s1 dynamic; s2 = diag (static)
                    rhs_off = [offs[0], offs[1], qb * 64]
                    rhs_off_b = [offs[2], offs[3], qb * 64]
                    for s_ in range(3):
                        cs = slice(p * 192 + s_ * 64, p * 192 + (s_ + 1) * 64)
                        nc.tensor.matmul(
                            sps[0:64, cs], lhsT=qT[p][0:64, qs],
                            rhs=kkT[p][0:64, ds(rhs_off[s_], 64)],
                            start=True, stop=True, skip_group_check=True)
                        nc.tensor.matmul(
                            sps[64:128, cs], lhsT=qTB[p][:, qs],
                            rhs=kkTB[p][:, ds(rhs_off_b[s_], 64)],
                            start=True, stop=True, skip_group_check=True,
                            tile_position=(0, 64))
                gi_state[0] = gi
                # exp (unnormalized, unmasked)
                pu = wrk.tile([128, 384], F16, tag="pu")
                nc.scalar.activation(pu[:], sps[:],
                                     mybir.ActivationFunctionType.Exp)
                # mask duplicated diag slot (x{1,0}, on Pool)
                for p in range(2):
                    nc.gpsimd.tensor_scalar(
                        pu[:, p * 192 + 128:p * 192 + 192],
                        pu[:, p * 192 + 128:p * 192 + 192],
                        wb[:, p * 32 + qb:p * 32 + qb + 1], None,
                        op0=mybir.AluOpType.mult)
                # rowsum + reciprocal
                den = wrk.tile([128, 2], F32, tag="den")
                for p in range(2):
                    nc.vector.tensor_reduce(
                        den[:, p:p + 1], pu[:, p * 192:(p + 1) * 192],
                        mybir.AxisListType.X, mybir.AluOpType.add)
                rden = wrk.tile([128, 2], F32, tag="rden")
                nc.vector.reciprocal(rden[:], den[:])
                # normalize on Pool (SBUF-only; Pool has no PSUM port)
                pr = wrk.tile([128, 384], F16, tag="pr")
                for p in range(2):
                    nc.gpsimd.tensor_scalar(
                        pr[:, p * 192:(p + 1) * 192],
                        pu[:, p * 192:(p + 1) * 192],
                        rden[:, p:p + 1], None, op0=mybir.AluOpType.mult)
                return pr, offs_p

            with tc.tile_pool(name="qkps", bufs=3, space="PSUM") as qkps, \
                 tc.tile_pool(name="vtps", bufs=2, space="PSUM") as vtps:
                # warm the PE p-state during the initial DMA wait: ~2.7us of
                # dummy transposes ramp the clock 0.65->2.4 GHz so the real
                # matmuls start at full speed
                for w in range(20):
                    tp = vtps.tile([64, 128], F16, name="tp")
                    nc.tensor.transpose(tp[:], ident[:, 0:64], ident[:])
                for t0, tw in XSL:
                    ts_ = slice(t0, t0 + tw)
                    last = t0 == 1536
                    for mt, dst_t in tgt:
                        qkv_group(qkps, ts_, mt, dst_t)
                        if last and mt == 3:
                            # kkT complete: replicate its head-B half, then
                            # pre-fill jj0's softmax chains so they overlap
                            # the q-projection tail (the PE queue is
                            # in-order, so emission position matters)
                            for p in range(2):
                                nc.sync.dma_start(kkTB[p][:, ts_],
                                                  kkT[p][64:128, ts_])
                            head_cache[0] = emit_head(0)
                            head_cache[1] = emit_head(1)
                        if mt == 5:
                            # v done for slice: transpose its blocks
                            for p in range(2):
                                for jb in range(tw // 64):
                                    j = t0 // 64 + jb
                                    tp = vtps.tile([64, 128], F16)
                                    nc.tensor.transpose(
                                        tp[:],
                                        vvT[p][:, j * 64:(j + 1) * 64],
                                        ident[:])
                                    vdst = vd[p][:, :, j * 64:(j + 1) * 64]
                                    src = tp[:].rearrange(
                                        "p (h x) -> p h x", h=2)
                                    if cp_state[0] % 2 == 0:
                                        nc.vector.tensor_copy(vdst, src)
                                    else:
                                        nc.scalar.copy(vdst, src)
                                    cp_state[0] += 1
                    # replicate head-B halves to partition base 0 (dynamic-
                    # offset matmul operands must have partition base 0)
                    for p in range(2):
                        if not last:
                            nc.sync.dma_start(kkTB[p][:, ts_],
                                              kkT[p][64:128, ts_])
                        nc.sync.dma_start(qTB[p][:, ts_],
                                          qT[p][64:128, ts_])

            # ---- block-sparse attention + interleaved output projection --
            # Both head-pairs share one softmax pass per query block:
            # sps/pu/pr are [128, 384] with pair p at cols p*192.
            with tc.tile_pool(name="ptps", bufs=2, space="PSUM") as ptps, \
                 tc.tile_pool(name="avps", bufs=1, space="PSUM") as avps, \
                 tc.tile_pool(name="otps", bufs=1, space="PSUM") as otps, \
                 tc.tile_pool(name="ypsp", bufs=1, space="PSUM") as ypsp:

                def emit_jj(jj):
                    avp2 = avps.tile([128, 256], F32, name="avp2")
                    avpP = [avp2[:, p * 128:(p + 1) * 128] for p in range(2)]
                    for qb2 in range(2):
                        qb = 2 * jj + qb2
                        if qb in head_cache:
                            pr, offs_p = head_cache.pop(qb)
                        else:
                            pr, offs_p = emit_head(qb)
                        # transpose probs per slot: [128q,64k]->[64k,128q]
                        pt = ptps.tile([64, 768], F16)
                        for p in range(2):
                            for s in range(3):
                                nc.tensor.transpose(
                                    pt[:, p * 384 + s * 128:
                                       p * 384 + (s + 1) * 128],
                                    pr[:, p * 192 + s * 64:
                                       p * 192 + (s + 1) * 64], ident[:])
                        pts = wrk.tile([64, 768], F16, tag="pts")
                        nc.vector.tensor_copy(pts[:], pt[:])
                        # AV accumulated over slots (dynamic rhs);
                        # qb even -> psum rows 0:64, odd -> 64:128
                        tp_ = None if qb2 == 0 else (0, 64)
                        for p in range(2):
                            avp = avpP[p][qb2 * 64:(qb2 + 1) * 64, :]
                            offs = offs_p[p]
                            ao = [offs[0], offs[1], qb * 64]
                            ao_b = [offs[2], offs[3], qb * 64]
                            for s in range(3):
                                nc.tensor.matmul(
                                    avp[:, 0:64],
                                    lhsT=pts[:, p * 384 + s * 128:
                                             p * 384 + s * 128 + 64],
                                    rhs=vd[p][:, 0, ds(ao[s], 64)],
                                    start=(s == 0), stop=(s == 2),
                                    skip_group_check=True,
                                    tile_position=tp_)
                            for s in range(3):
                                mi = nc.tensor.matmul(
                                    avp[:, 64:128],
                                    lhsT=pts[:, p * 384 + s * 128 + 64:
                                             p * 384 + s * 128 + 128],
                                    rhs=vd[p][:, 1, ds(ao_b[s], 64)],
                                    start=(s == 0), stop=(s == 2),
                                    skip_group_check=True,
                                    tile_position=tp_)
                            anchors.append(mi)
                    # both qbs done: copy AV pairs, transpose
                    # [128tok,128feat] -> [128feat,128tok] per pair
                    otp = otps.tile([128, 256], F16)
                    for p in range(2):
                        av_sb = wrk.tile([128, 128], F16, tag="av_sb")
                        nc.scalar.copy(av_sb[:], avpP[p])
                        nc.tensor.transpose(
                            otp[:, p * 128:(p + 1) * 128], av_sb[:],
                            ident[:])
                    nc.vector.tensor_copy(
                        outT[0][:, jj * 128:(jj + 1) * 128], otp[:, 0:128])
                    nc.vector.tensor_copy(
                        outT[1][:, jj * 128:(jj + 1) * 128], otp[:, 128:256])
                    # output projection for this 128-token tile
                    tks = slice(jj * 128, (jj + 1) * 128)
                    if jj < 15:
                        for half in range(2):
                            ns = slice(half * 512, (half + 1) * 512)
                            yp = ypsp.tile([128, 512], F32)
                            nc.tensor.matmul(yp[:], lhsT=outT[0][:, tks],
                                             rhs=pwt[:, 0, ns],
                                             start=True, stop=False)
                            nc.tensor.matmul(yp[:], lhsT=outT[1][:, tks],
                                             rhs=pwt[:, 1, ns],
                                             start=False, stop=True)
                            ys = wrk.tile([128, 512], F16, tag="ys")
                            nc.scalar.copy(ys[:], yp[:])
                            nc.sync.dma_start(y_d[tks, ns], ys[:])
                    else:
                        # drain: quarter the last tile across two PSUM pools
                        # so the copies/DMAs pipeline on both DVE and Act
                        for q in range(4):
                            ns = slice(q * 256, (q + 1) * 256)
                            if q % 2 == 0:
                                yp = ypsp.tile([128, 512], F32,
                                               name="yp")[:, 0:256]
                            else:
                                yp = avps.tile([128, 256], F32, name="avp2")
                                yp = yp[:, 0:256]
                            nc.tensor.matmul(yp, lhsT=outT[0][:, tks],
                                             rhs=pwt[:, 0, ns],
                                             start=True, stop=False)
                            nc.tensor.matmul(yp, lhsT=outT[1][:, tks],
                                             rhs=pwt[:, 1, ns],
                                             start=False, stop=True)
                            ys = wrk.tile([128, 512], F16, tag="ys")
                            if q % 2 == 0:
                                nc.scalar.copy(ys[:, 0:256], yp)
                            else:
                                nc.vector.tensor_copy(ys[:, 0:256], yp)
                            nc.sync.dma_start(y_d[tks, ns], ys[:, 0:256])

                for jj in range(16):
                    emit_jj(jj)
            spsp_ctx.__exit__(None, None, None)

    nc.finalize()
    return nc


def _host_prep(x, qkv_w, proj_w):
    """Per-core input maps + block selection (float64, matches fp32 ref)."""
    in_maps = []
    x64 = x.astype(np.float64)
    for core in range(NCORES):
        b = core // (NCORES // B)
        hg = core % (NCORES // B)
        heads = [hg * HPC + i for i in range(HPC)]

        xt = np.ascontiguousarray(x[b].T).astype(np.float16)

        wqkvT = np.empty((DIM, 768), np.float32)
        for p in range(2):
            hA, hB = heads[2 * p], heads[2 * p + 1]
            # q: cols [p*128, p*128+128); k: 256 + p*128; v: 512 + p*128
            wqkvT[:, p*128:p*128+64] = qkv_w[hA*64:(hA+1)*64].T * SCALE
            wqkvT[:, p*128+64:p*128+128] = qkv_w[hB*64:(hB+1)*64].T * SCALE
            kbase = 256 + p * 128
            wqkvT[:, kbase:kbase+64] = qkv_w[DIM+hA*64:DIM+(hA+1)*64].T
            wqkvT[:, kbase+64:kbase+128] = qkv_w[DIM+hB*64:DIM+(hB+1)*64].T
            vbase = 512 + p * 128
            wqkvT[:, vbase:vbase+64] = qkv_w[2*DIM+hA*64:2*DIM+(hA+1)*64].T
            wqkvT[:, vbase+64:vbase+128] = qkv_w[2*DIM+hB*64:2*DIM+(hB+1)*64].T
        wqkvT = wqkvT.astype(np.float16)

        pw = np.ascontiguousarray(
            proj_w[:, heads[0]*64:(heads[-1]+1)*64].T).astype(np.float16)

        # float64 selection (matches fp32 reference ordering w/ margin)
        xb = x64[b].reshape(M, BS, DIM).mean(axis=1)
        selidx = np.zeros((1, 256), np.int32)
        wbias = np.ones((128, 64), np.float32)
        for p in range(2):
            for hip in range(2):
                h = heads[2 * p + hip]
                qb_ = xb @ qkv_w[h*64:(h+1)*64].T.astype(np.float64)
                kb_ = xb @ qkv_w[DIM+h*64:DIM+(h+1)*64].T.astype(np.float64)
                c = qb_ @ kb_.T
                for i in range(M):
                    order = np.argsort(-c[i], kind="stable")
                    i1, i2 = int(order[0]), int(order[1])
                    col = p * 128 + i * 4 + hip * 2
                    selidx[0, col] = i1 * 64
                    selidx[0, col + 1] = i2 * 64
                    if i == i1 or i == i2:
                        wbias[hip*64:(hip+1)*64, p*32+i] = 0.0
        in_maps.append({"xt": xt, "wq": wqkvT, "pw": pw,
                        "selidx": selidx, "wbias": wbias})
    return in_maps


def kernel(x, qkv_w, proj_w, proj_b):
    global _NC_CACHE, LAST_RESULTS
    x = np.asarray(x, np.float32)
    qkv_w = np.asarray(qkv_w, np.float32)
    proj_w = np.asarray(proj_w, np.float32)
    proj_b = np.asarray(proj_b, np.float32)

    if _NC_CACHE is None:
        _NC_CACHE = build_kernel()
    nc = _NC_CACHE

    in_maps = _host_prep(x, qkv_w, proj_w)
    res = run_bass_kernel_spmd(nc, in_maps, list(range(NCORES)))
    LAST_RESULTS = res

    out = np.zeros((B, N, DIM), np.float32)
    for core in range(NCORES):
        out[core // (NCORES // B)] += res.results[core]["y"].astype(np.float32)
    out += proj_b[None, None, :]
    return out


# revision 84
# speedup vs baseline: 1.0394x; 1.0065x over previous
"""Block-sparse attention (CABAttention) Trainium2 kernel.

Sharding: 8 cores = 2 batches x 4 head-groups (4 heads each).
Per core: qkv projection (fp16), top-2+diag block-sparse attention
(fp16 values path, fp32 PSUM/softmax-denominator), output projection.
Block selection (top-2 of coarse block-mean scores) is computed on host
in float64 (the PE's fp16 matmul error would flip near-tied blocks)
and passed as index inputs, consumed on device via dynamic
access-pattern offsets on the matmul moving operand.
Unshard: partial output projections (fp16) summed over the 4 cores per
batch (row-parallel tensor split) in fp32 on host, plus bias.

Engine budget (vs the previous fp32r version): input DMAs are token-
sliced and dual-queued so the first qkv matmul group is ready ~4us in;
the v key-major transpose is interleaved into the qkv phase; PSUM->SBUF
copies are split across DVE and Act; the softmax normalize runs on the
(otherwise idle) Pool engine, which has no PSUM port but this multiply
is SBUF-only; the output projection is interleaved into the attention
loop per 128-token tile and written as fp16.
"""
import sys

sys.path.insert(0, "/opt/trn_rl_repo")

import numpy as np

import concourse.bass as bass
import concourse.mybir as mybir
import concourse.tile as tile
from concourse import bacc
from concourse.bass import ds
from concourse.bass_utils import run_bass_kernel_spmd
from concourse.masks import make_identity

F32 = mybir.dt.float32
F16 = mybir.dt.float16
I32 = mybir.dt.int32

DIM = 1024
H = 16
HD = 64
BS = 64
N = 2048
B = 2
M = N // BS            # 32 blocks
SCALE = HD ** -0.5
NCORES = 8
HPC = H // (NCORES // B)   # 4 heads per core

_NC_CACHE = None
LAST_RESULTS = None


def build_kernel():
    nc = bacc.Bacc(None)
    xt_d = nc.dram_tensor("xt", [DIM, N], F16, kind="ExternalInput")
    wq_d = nc.dram_tensor("wq", [DIM, 768], F16, kind="ExternalInput")
    pw_d = nc.dram_tensor("pw", [256, DIM], F16, kind="ExternalInput")
    idx_d = nc.dram_tensor("selidx", [1, 256], I32, kind="ExternalInput")
    wb_d = nc.dram_tensor("wbias", [128, 64], F32, kind="ExternalInput")
    y_d = nc.dram_tensor("y", [N, DIM], F16, kind="ExternalOutput")

    with tile.TileContext(nc) as tc:
        with tc.tile_pool(name="big", bufs=1) as big, \
             tc.tile_pool(name="wrk", bufs=6) as wrk:

            # ---- persistent SBUF tensors ----
            xt = big.tile([128, 8, N], F16)           # x^T, feature-major
            wq = big.tile([128, 8, 768], F16)         # qkv weights^T
            pwt = big.tile([128, 2, DIM], F16)        # proj weights
            idx = big.tile([1, 256], I32)
            wb = big.tile([128, 64], F32)
            qT = [big.tile([128, N], F16, name=f"qT{i}") for i in range(2)]
            kkT = [big.tile([128, N], F16, name=f"kkT{i}") for i in range(2)]
            vvT = [big.tile([128, N], F16, name=f"vvT{i}") for i in range(2)]
            vd = [big.tile([64, 2, N], F16, name=f"vd{i}") for i in range(2)]
            outT = [big.tile([128, N], F16, name=f"outT{i}") for i in range(2)]
            qTB = [big.tile([64, N], F16, name=f"qTB{i}") for i in range(2)]
            kkTB = [big.tile([64, N], F16, name=f"kkTB{i}") for i in range(2)]
            identf = big.tile([128, 128], F32)
            ident = big.tile([128, 128], F16)

            make_identity(nc, identf[:])
            nc.vector.tensor_copy(ident[:], identf[:])

            # ---- input DMAs, token-sliced for early compute start ----
            xt_v = xt_d[:].rearrange("(a p) n -> p a n", p=128)
            wq_v = wq_d[:].rearrange("(a p) n -> p a n", p=128)
            pw_v = pw_d[:].rearrange("(a p) n -> p a n", p=128)

            # qkv weights in three 256-col blocks, v first to match the
            # v -> k -> q matmul order below.
            XSL = [(0, 512), (512, 512), (1024, 512), (1536, 512)]
            nc.sync.dma_start(wq[:, :, 512:768], wq_v[:, :, 512:768])
            for si, (t0, tw) in enumerate(XSL):
                ts_ = slice(t0, t0 + tw)
                for k in range(8):
                    eng = nc.sync if k % 2 == 0 else nc.scalar
                    eng.dma_start(xt[:, k, ts_], xt_v[:, k, ts_])
                if si == 0:
                    nc.sync.dma_start(wq[:, :, 256:512], wq_v[:, :, 256:512])
                elif si == 1:
                    nc.sync.dma_start(wq[:, :, 0:256], wq_v[:, :, 0:256])
            nc.sync.dma_start(idx[:], idx_d[:])
            nc.sync.dma_start(wb[:], wb_d[:])
            nc.scalar.dma_start(pwt[:], pw_v[:])

            # ---- qkv projection + v key-major transpose, per token slice --
            # v first, then k, then q (matches the weight DMA order above).
            # wq col blocks: q at mt 0,1; k at 2,3; v at 4,5.
            tgt = [(4, vvT[0]), (5, vvT[1]), (2, kkT[0]), (3, kkT[1]),
                   (0, qT[0]), (1, qT[1])]
            cp_state = [0]

            def qkv_group(pool, ts_, mt, dst_t):
                ps = pool.tile([128, 512], F32, name="yp")
                pv = ps[:, 0:ts_.stop - ts_.start]
                for k in range(8):
                    nc.tensor.matmul(
                        pv, lhsT=wq[:, k, mt * 128:(mt + 1) * 128],
                        rhs=xt[:, k, ts_],
                        start=(k == 0), stop=(k == 7))
                if cp_state[0] % 2 == 0:
                    nc.vector.tensor_copy(dst_t[:, ts_], pv)
                else:
                    nc.scalar.copy(dst_t[:, ts_], pv)
                cp_state[0] += 1

            anchors = []
            gi_state = [0]
            head_cache = {}
            spsp_ctx = tc.tile_pool(name="spsp", bufs=3, space="PSUM")
            spsp = spsp_ctx.__enter__()

            def emit_head(qb):
                # scores -> exp -> mask -> rowsum -> recip -> normalize;
                # touches only spsp + SBUF, so jj0's heads can be emitted
                # mid qkv-phase to pre-fill the softmax pipeline.
                gi = gi_state[0]
                qs = slice(qb * 64, (qb + 1) * 64)
                sps = spsp.tile([128, 384], F32, name="sps")
                offs_p = []
                for p in range(2):
                    base = p * 128 + qb * 4
                    eng = nc.tensor
                    tmps = [eng.alloc_register(f"off_{base + c}")
                            for c in range(4)]
                    li = eng.reg_load(tmps, idx[0:1, base:base + 4])
                    if gi >= 13:
                        tile.add_dep_helper(
                            li.ins, anchors[gi - 13].ins, sync=False,
                            reason="bound PE register live range")
                    offs = [eng.snap(t_, donate=True, min_val=0,
                                     max_val=N - 64) for t_ in tmps]
                    offs_p.append(offs)
                    gi += 1
                    # scores: slots s0,s1 dynamic; s2 = diag (static)
                    rhs_off = [offs[0], offs[1], qb * 64]
                    rhs_off_b = [offs[2], offs[3], qb * 64]
                    for s_ in range(3):
                        cs = slice(p * 192 + s_ * 64, p * 192 + (s_ + 1) * 64)
                        nc.tensor.matmul(
                            sps[0:64, cs], lhsT=qT[p][0:64, qs],
                            rhs=kkT[p][0:64, ds(rhs_off[s_], 64)],
                            start=True, stop=True, skip_group_check=True)
                        nc.tensor.matmul(
                            sps[64:128, cs], lhsT=qTB[p][:, qs],
                            rhs=kkTB[p][:, ds(rhs_off_b[s_], 64)],
                            start=True, stop=True, skip_group_check=True,
                            tile_position=(0, 64))
                gi_state[0] = gi
                # exp (unnormalized, unmasked)
                pu = wrk.tile([128, 384], F16, tag="pu")
                nc.scalar.activation(pu[:], sps[:],
                                     mybir.ActivationFunctionType.Exp)
                # mask duplicated diag slot (x{1,0}, on Pool)
                for p in range(2):
                    nc.gpsimd.tensor_scalar(
                        pu[:, p * 192 + 128:p * 192 + 192],
                        pu[:, p * 192 + 128:p * 192 + 192],
                        wb[:, p * 32 + qb:p * 32 + qb + 1], None,
                        op0=mybir.AluOpType.mult)
                # rowsum + reciprocal
                den = wrk.tile([128, 2], F32, tag="den")
                for p in range(2):
                    nc.vector.tensor_reduce(
                        den[:, p:p + 1], pu[:, p * 192:(p + 1) * 192],
                        mybir.AxisListType.X, mybir.AluOpType.add)
                rden = wrk.tile([128, 2], F32, tag="rden")
                nc.vector.reciprocal(rden[:], den[:])
                # normalize on Pool (SBUF-only; Pool has no PSUM port)
                pr = wrk.tile([128, 384], F16, tag="pr")
                for p in range(2):
                    nc.gpsimd.tensor_scalar(
                        pr[:, p * 192:(p + 1) * 192],
                        pu[:, p * 192:(p + 1) * 192],
                        rden[:, p:p + 1], None, op0=mybir.AluOpType.mult)
                return pr, offs_p

            with tc.tile_pool(name="qkps", bufs=3, space="PSUM") as qkps, \
                 tc.tile_pool(name="vtps", bufs=2, space="PSUM") as vtps:
                # warm the PE p-state during the initial DMA wait: ~2.7us of
                # dummy transposes ramp the clock 0.65->2.4 GHz so the real
                # matmuls start at full speed
                for w in range(20):
                    tp = vtps.tile([64, 128], F16, name="tp")
                    nc.tensor.transpose(tp[:], ident[:, 0:64], ident[:])
                for t0, tw in XSL:
                    ts_ = slice(t0, t0 + tw)
                    last = t0 == 1536
                    for mt, dst_t in tgt:
                        qkv_group(qkps, ts_, mt, dst_t)
                        if last and mt == 3:
                            # kkT complete: replicate its head-B half, then
                            # pre-fill jj0's softmax chains so they overlap
                            # the q-projection tail (the PE queue is
                            # in-order, so emission position matters)
                            for p in range(2):
                                nc.sync.dma_start(kkTB[p][:, ts_],
                                                  kkT[p][64:128, ts_])
                            head_cache[0] = emit_head(0)
                            head_cache[1] = emit_head(1)
                        if mt == 5:
                            # v done for slice: transpose its blocks
                            for p in range(2):
                                for jb in range(tw // 64):
                                    j = t0 // 64 + jb
                                    tp = vtps.tile([64, 128], F16)
                                    nc.tensor.transpose(
                                        tp[:],
                                        vvT[p][:, j * 64:(j + 1) * 64],
                                        ident[:])
                                    vdst = vd[p][:, :, j * 64:(j + 1) * 64]
                                    src = tp[:].rearrange(
                                        "p (h x) -> p h x", h=2)
                                    if cp_state[0] % 2 == 0:
                                        nc.vector.tensor_copy(vdst, src)
                                    else:
                                        nc.scalar.copy(vdst, src)
                                    cp_state[0] += 1
                    # replicate head-B halves to partition base 0 (dynamic-
                    # offset matmul operands must have partition base 0)
                    for p in range(2):
                        if not last:
                            nc.sync.dma_start(kkTB[p][:, ts_],
                                              kkT[p][64:128, ts_])
                        nc.sync.dma_start(qTB[p][:, ts_],
                                          qT[p][64:128, ts_])

            # ---- block-sparse attention + interleaved output projection --
            # Both head-pairs share one softmax pass per query block:
            # sps/pu/pr are [128, 384] with pair p at cols p*192.
            with tc.tile_pool(name="ptps", bufs=2, space="PSUM") as ptps, \
                 tc.tile_pool(name="avps", bufs=1, space="PSUM") as avps, \
                 tc.tile_pool(name="otps", bufs=1, space="PSUM") as otps, \
                 tc.tile_pool(name="ypsp", bufs=1, space="PSUM") as ypsp:

                def emit_jj(jj):
                    avp2 = avps.tile([128, 256], F32, name="avp2")
                    avpP = [avp2[:, p * 128:(p + 1) * 128] for p in range(2)]
                    for qb2 in range(2):
                        qb = 2 * jj + qb2
                        if qb in head_cache:
                            pr, offs_p = head_cache.pop(qb)
                        else:
                            pr, offs_p = emit_head(qb)
                        # transpose probs per slot: [128q,64k]->[64k,128q]
                        pt = ptps.tile([64, 768], F16)
                        for p in range(2):
                            for s in range(3):
                                nc.tensor.transpose(
                                    pt[:, p * 384 + s * 128:
                                       p * 384 + (s + 1) * 128],
                                    pr[:, p * 192 + s * 64:
                                       p * 192 + (s + 1) * 64], ident[:])
                        pts = wrk.tile([64, 768], F16, tag="pts")
                        nc.vector.tensor_copy(pts[:], pt[:])
                        # AV accumulated over slots (dynamic rhs);
                        # qb even -> psum rows 0:64, odd -> 64:128
                        tp_ = None if qb2 == 0 else (0, 64)
                        for p in range(2):
                            avp = avpP[p][qb2 * 64:(qb2 + 1) * 64, :]
                            offs = offs_p[p]
                            ao = [offs[0], offs[1], qb * 64]
                            ao_b = [offs[2], offs[3], qb * 64]
                            for s in range(3):
                                nc.tensor.matmul(
                                    avp[:, 0:64],
                                    lhsT=pts[:, p * 384 + s * 128:
                                             p * 384 + s * 128 + 64],
                                    rhs=vd[p][:, 0, ds(ao[s], 64)],
                                    start=(s == 0), stop=(s == 2),
                                    skip_group_check=True,
                                    tile_position=tp_)
                            for s in range(3):
                                mi = nc.tensor.matmul(
                                    avp[:, 64:128],
                                    lhsT=pts[:, p * 384 + s * 128 + 64:
                                             p * 384 + s * 128 + 128],
                                    rhs=vd[p][:, 1, ds(ao_b[s], 64)],
                                    start=(s == 0), stop=(s == 2),
                                    skip_group_check=True,
                                    tile_position=tp_)
                            anchors.append(mi)
                    # both qbs done: copy AV pairs, transpose
                    # [128tok,128feat] -> [128feat,128tok] per pair
                    otp = otps.tile([128, 256], F16)
                    for p in range(2):
                        av_sb = wrk.tile([128, 128], F16, tag="av_sb")
                        if jj == 15 and p == 1:
                            nc.vector.tensor_copy(av_sb[:], avpP[p])
                        else:
                            nc.scalar.copy(av_sb[:], avpP[p])
                        nc.tensor.transpose(
                            otp[:, p * 128:(p + 1) * 128], av_sb[:],
                            ident[:])
                    if jj == 15:
                        nc.scalar.copy(
                            outT[1][:, jj * 128:(jj + 1) * 128],
                            otp[:, 128:256])
                    else:
                        nc.vector.tensor_copy(
                            outT[1][:, jj * 128:(jj + 1) * 128],
                            otp[:, 128:256])
                    nc.vector.tensor_copy(
                        outT[0][:, jj * 128:(jj + 1) * 128], otp[:, 0:128])
                    # output projection for this 128-token tile
                    tks = slice(jj * 128, (jj + 1) * 128)
                    if jj < 15:
                        for half in range(2):
                            ns = slice(half * 512, (half + 1) * 512)
                            yp = ypsp.tile([128, 512], F32)
                            nc.tensor.matmul(yp[:], lhsT=outT[0][:, tks],
                                             rhs=pwt[:, 0, ns],
                                             start=True, stop=False)
                            nc.tensor.matmul(yp[:], lhsT=outT[1][:, tks],
                                             rhs=pwt[:, 1, ns],
                                             start=False, stop=True)
                            ys = wrk.tile([128, 512], F16, tag="ys")
                            nc.scalar.copy(ys[:], yp[:])
                            nc.sync.dma_start(y_d[tks, ns], ys[:])
                    else:
                        # drain: quarter the last tile across two PSUM pools
                        # so the copies/DMAs pipeline on both DVE and Act
                        for q in range(4):
                            ns = slice(q * 256, (q + 1) * 256)
                            if q == 0:
                                yp = ypsp.tile([128, 512], F32,
                                               name="yp")[:, 0:256]
                            elif q == 1:
                                yp = avps.tile([128, 256], F32,
                                               name="avp2")[:, 0:256]
                            else:
                                # score-PSUM slots are free at the drain
                                yp = spsp.tile([128, 384], F32,
                                               name="sps")[:, 0:256]
                            nc.tensor.matmul(yp, lhsT=outT[0][:, tks],
                                             rhs=pwt[:, 0, ns],
                                             start=True, stop=False)
                            nc.tensor.matmul(yp, lhsT=outT[1][:, tks],
                                             rhs=pwt[:, 1, ns],
                                             start=False, stop=True)
                            ys = wrk.tile([128, 512], F16, tag="ys")
                            if q % 2 == 0:
                                nc.scalar.copy(ys[:, 0:256], yp)
                                nc.sync.dma_start(y_d[tks, ns], ys[:, 0:256])
                            else:
                                nc.vector.tensor_copy(ys[:, 0:256], yp)
                                nc.scalar.dma_start(y_d[tks, ns],
                                                    ys[:, 0:256])

                for jj in range(16):
                    emit_jj(jj)
            spsp_ctx.__exit__(None, None, None)

    nc.finalize()
    return nc


def _host_prep(x, qkv_w, proj_w):
    """Per-core input maps + block selection (float64, matches fp32 ref)."""
    in_maps = []
    x64 = x.astype(np.float64)
    for core in range(NCORES):
        b = core // (NCORES // B)
        hg = core % (NCORES // B)
        heads = [hg * HPC + i for i in range(HPC)]

        xt = np.ascontiguousarray(x[b].T).astype(np.float16)

        wqkvT = np.empty((DIM, 768), np.float32)
        for p in range(2):
            hA, hB = heads[2 * p], heads[2 * p + 1]
            # q: cols [p*128, p*128+128); k: 256 + p*128; v: 512 + p*128
            wqkvT[:, p*128:p*128+64] = qkv_w[hA*64:(hA+1)*64].T * SCALE
            wqkvT[:, p*128+64:p*128+128] = qkv_w[hB*64:(hB+1)*64].T * SCALE
            kbase = 256 + p * 128
            wqkvT[:, kbase:kbase+64] = qkv_w[DIM+hA*64:DIM+(hA+1)*64].T
            wqkvT[:, kbase+64:kbase+128] = qkv_w[DIM+hB*64:DIM+(hB+1)*64].T
            vbase = 512 + p * 128
            wqkvT[:, vbase:vbase+64] = qkv_w[2*DIM+hA*64:2*DIM+(hA+1)*64].T
            wqkvT[:, vbase+64:vbase+128] = qkv_w[2*DIM+hB*64:2*DIM+(hB+1)*64].T
        wqkvT = wqkvT.astype(np.float16)

        pw = np.ascontiguousarray(
            proj_w[:, heads[0]*64:(heads[-1]+1)*64].T).astype(np.float16)

        # float64 selection (matches fp32 reference ordering w/ margin)
        xb = x64[b].reshape(M, BS, DIM).mean(axis=1)
        selidx = np.zeros((1, 256), np.int32)
        wbias = np.ones((128, 64), np.float32)
        for p in range(2):
            for hip in range(2):
                h = heads[2 * p + hip]
                qb_ = xb @ qkv_w[h*64:(h+1)*64].T.astype(np.float64)
                kb_ = xb @ qkv_w[DIM+h*64:DIM+(h+1)*64].T.astype(np.float64)
                c = qb_ @ kb_.T
                for i in range(M):
                    order = np.argsort(-c[i], kind="stable")
                    i1, i2 = int(order[0]), int(order[1])
                    col = p * 128 + i * 4 + hip * 2
                    selidx[0, col] = i1 * 64
                    selidx[0, col + 1] = i2 * 64
                    if i == i1 or i == i2:
                        wbias[hip*64:(hip+1)*64, p*32+i] = 0.0
        in_maps.append({"xt": xt, "wq": wqkvT, "pw": pw,
                        "selidx": selidx, "wbias": wbias})
    return in_maps


def kernel(x, qkv_w, proj_w, proj_b):
    global _NC_CACHE, LAST_RESULTS
    x = np.asarray(x, np.float32)
    qkv_w = np.asarray(qkv_w, np.float32)
    proj_w = np.asarray(proj_w, np.float32)
    proj_b = np.asarray(proj_b, np.float32)

    if _NC_CACHE is None:
        _NC_CACHE = build_kernel()
    nc = _NC_CACHE

    in_maps = _host_prep(x, qkv_w, proj_w)
    res = run_bass_kernel_spmd(nc, in_maps, list(range(NCORES)))
    LAST_RESULTS = res

    out = np.zeros((B, N, DIM), np.float32)
    for core in range(NCORES):
        out[core // (NCORES // B)] += res.results[core]["y"].astype(np.float32)
    out += proj_b[None, None, :]
    return out
